# revision 1
# baseline (speedup 1.0000x reference)
"""Trainium2 Bass kernel for nn_CausalLayer (gnn_message_passing).

Reference computation:
    sigma = mu + eps * exp(logstd)                      [B*N_PER, D]
    M     = pinv(I - A.T)                               [N, N]  (well-conditioned -> inv)
    sub_b = M[idx_b, :][:, idx_b]                       [128, 128] per graph
    z_b   = sub_b @ sigma_b                             [128, D] per graph
    return (sigma, z)

Distribution: data-parallel over graphs, 256 graphs per core x 8 cores.
M (the inverse) is computed once on the host, replicated to every core.

Device pipeline per core (per 16-graph window):
  - elementwise sigma stream in DMA-contiguous "giant" tiles [128, 1024] f32
  - gather stage 1: SBUF-source dma_gather (transpose) pulls the 128 needed
    columns of M per graph -> C[p, m, i] = M[128m+p, idx_b[i]]  (bf16)
  - gather stage 2: SBUF-source dma_gather (transpose) over C's rank-stripe
    layout pulls rows idx_b -> subT[j, i] = M[idx_i, idx_j]  (bf16), which is
    exactly the matmul lhsT layout
  - per graph: PSUM z = subT.T @ sig_b (bf16 x bf16 -> f32)
"""

import sys

for _p in ("/opt/trn_rl_repo",):
    if _p not in sys.path:
        sys.path.insert(0, _p)

import numpy as np
import ml_dtypes

import concourse.bass as bass
import concourse.mybir as mybir
from concourse import bacc
from concourse.tile import TileContext

N_GLOBAL = 1024
B = 2048
N_PER = 128
D = 64
N_CORES = 8
B_CORE = B // N_CORES            # 256 graphs per core
ROWS_CORE = B_CORE * N_PER       # 32768 rows per core
GW = 16                          # graphs per window
N_WIN = B_CORE // GW             # 16 windows per core
EW_FREE = GW * N_PER * D // 128  # 1024 f32 per partition per window tile


# ---------------------------------------------------------------------------
# Workaround: this walrus build accepts only one sync-wait per instruction
# (setupSyncWait "Too many sync wait commands").  After Tile scheduling,
# split any instruction's excess waits onto preceding NoOps on its engine --
# the engine blocks on the NoOps' waits first, so semantics are unchanged.
# ---------------------------------------------------------------------------
MAX_WAITS_PER_INST = 1


def _split_sync_waits(nc: bass.Bass, max_waits: int = MAX_WAITS_PER_INST):
    for fn in nc.m.functions:
        for blk in fn.blocks:
            insts = list(blk.instructions)
            out = []
            changed = False
            for inst in insts:
                si = inst.sync_info
                waits = list(si.on_wait) if si is not None and si.on_wait else []
                if len(waits) > max_waits:
                    changed = True
                    keep = waits[-max_waits:]
                    extra = waits[: len(waits) - max_waits]
                    for k, w in enumerate(extra):
                        nop = mybir.InstNoOp(
                            name=f"{inst.name}-ws{k}",
                            engine=inst.engine,
                            ins=[],
                            outs=[],
                            sync_info=mybir.SyncInfo(on_wait=[w], on_update=[]),
                        )
                        nc.register_instruction(nop)
                        out.append(nop)
                    inst.sync_info = mybir.SyncInfo(
                        on_wait=keep,
                        on_update=list(si.on_update) if si.on_update else [],
                    )
                out.append(inst)
            if changed:
                blk.instructions = out


# ---------------------------------------------------------------------------
# Device program (identical on all 8 cores; all per-core data arrives as
# DRAM inputs).
# ---------------------------------------------------------------------------
def build_program() -> bass.Bass:
    nc = bacc.Bacc(num_swdge_queues=1, dynamic_dma_scratch_size=32768)
    f32 = mybir.dt.float32
    bf16 = mybir.dt.bfloat16
    i16 = mybir.dt.int16

    # "giant" layout: [win, 128, 1024]; partition p holds rows
    # [2048*win + 16*p, +16) of the [32768, 64] row-major shard.
    mu = nc.dram_tensor("mu", [N_WIN, 128, EW_FREE], f32, kind="ExternalInput")
    ls = nc.dram_tensor("logstd", [N_WIN, 128, EW_FREE], f32, kind="ExternalInput")
    eps = nc.dram_tensor("eps", [N_WIN, 128, EW_FREE], f32, kind="ExternalInput")
    mpack = nc.dram_tensor("mpack", [128, 8192], bf16, kind="ExternalInput")
    idx1 = nc.dram_tensor("idx1", [N_WIN, 128, 128], i16, kind="ExternalInput")
    idx2 = nc.dram_tensor("idx2", [N_WIN, 128, 128], i16, kind="ExternalInput")

    sigma = nc.dram_tensor("sigma", [N_WIN, 128, EW_FREE], f32, kind="ExternalOutput")
    z = nc.dram_tensor("z", [B_CORE, N_PER, D], f32, kind="ExternalOutput")

    with TileContext(nc) as tc:
        with (
            tc.tile_pool(name="const", bufs=1) as constp,
            tc.tile_pool(name="ew", bufs=2) as ewp,
            tc.tile_pool(name="sigr", bufs=2) as sigrp,
            tc.tile_pool(name="gath", bufs=2) as gathp,
            tc.tile_pool(name="idxp", bufs=2) as idxp,
            tc.tile_pool(name="zsb", bufs=4) as zsbp,
            tc.tile_pool(name="zps", bufs=4, space="PSUM") as zpsp,
        ):
            msb = constp.tile([128, 8192], bf16)
            nc.sync.dma_start(out=msb[:, :], in_=mpack[:, :])

            live = {}

            def emit_front(w):
                """ew stream + idx loads + stage-1 gathers for window w."""
                mu_t = ewp.tile([128, EW_FREE], f32, tag="mu")
                ls_t = ewp.tile([128, EW_FREE], f32, tag="ls")
                eps_t = ewp.tile([128, EW_FREE], f32, tag="eps")
                nc.sync.dma_start(out=mu_t[:, :], in_=mu[w])
                nc.sync.dma_start(out=ls_t[:, :], in_=ls[w])
                nc.sync.dma_start(out=eps_t[:, :], in_=eps[w])

                tmp_t = ewp.tile([128, EW_FREE], f32, tag="tmp")
                sig_t = ewp.tile([128, EW_FREE], f32, tag="sig")
                nc.scalar.activation(
                    tmp_t[:, :], ls_t[:, :], mybir.ActivationFunctionType.Exp
                )
                nc.vector.tensor_mul(tmp_t[:, :], tmp_t[:, :], eps_t[:, :])
                nc.vector.tensor_add(sig_t[:, :], tmp_t[:, :], mu_t[:, :])
                nc.sync.dma_start(out=sigma[w], in_=sig_t[:, :])

                sigb_t = ewp.tile([128, EW_FREE], bf16, tag="sigb")
                nc.vector.tensor_copy(sigb_t[:, :], sig_t[:, :])

                sigr_t = sigrp.tile([128, GW * D], bf16)
                for g in range(GW):
                    src = sigb_t[8 * g : 8 * g + 8, :].rearrange(
                        "p (jl d) -> p jl d", d=D
                    )
                    dst = sigr_t[:, D * g : D * (g + 1)]
                    eng = nc.scalar if (g % 2 == 0) else nc.sync
                    eng.dma_start(out=dst, in_=src)

                i1_t = idxp.tile([128, 128], i16, tag="i1")
                i2_t = idxp.tile([128, 128], i16, tag="i2")
                nc.sync.dma_start(out=i1_t[:, :], in_=idx1[w])
                nc.sync.dma_start(out=i2_t[:, :], in_=idx2[w])

                c_t = gathp.tile([128, 8 * 128 * GW], bf16, tag="c")
                for cc in range(4):
                    nc.gpsimd.dma_gather(
                        out_ap=c_t[:, 4096 * cc : 4096 * (cc + 1)].rearrange(
                            "p (m i) -> p m i", m=8
                        ),
                        in_ap=msb[:, :],
                        idxs_ap=i1_t[:, 32 * cc : 32 * (cc + 1)],
                        num_idxs=512,
                        num_idxs_reg=512,
                        elem_size=1024,
                        transpose=True,
                        queue_num=0,
                        sbuf_tokens_per_rank=128,
                        sbuf_free_dim_per_rank=2048,
                        sbuf_free_dim_pad_per_rank=0,
                        sbuf_byte_offset=0,
                    )
                live[w] = (c_t, i2_t, sigr_t)

            def emit_back(w):
                """stage-2 gathers + matmuls + z out for window w."""
                c_t, i2_t, sigr_t = live.pop(w)
                subt_t = gathp.tile([128, 128 * GW], bf16, tag="subt")
                for cc in range(4):
                    nc.gpsimd.dma_gather(
                        out_ap=subt_t[:, 512 * cc : 512 * (cc + 1)].rearrange(
                            "p (o i) -> p o i", o=1
                        ),
                        in_ap=c_t[:, :],
                        idxs_ap=i2_t[:, 32 * cc : 32 * (cc + 1)],
                        num_idxs=512,
                        num_idxs_reg=512,
                        elem_size=128,
                        transpose=True,
                        queue_num=0,
                        sbuf_tokens_per_rank=128,
                        sbuf_free_dim_per_rank=256,
                        sbuf_free_dim_pad_per_rank=0,
                        sbuf_byte_offset=0,
                    )
                for half in range(2):
                    zps_t = zpsp.tile([128, 8 * D], mybir.dt.float32)
                    for gg in range(8):
                        g = 8 * half + gg
                        nc.tensor.matmul(
                            zps_t[:, D * gg : D * (gg + 1)],
                            subt_t[:, 128 * g : 128 * (g + 1)],
                            sigr_t[:, D * g : D * (g + 1)],
                            start=True,
                            stop=True,
                        )
                    zsb_t = zsbp.tile([128, 8 * D], mybir.dt.float32)
                    nc.scalar.copy(out=zsb_t[:, :], in_=zps_t[:, :])
                    b0 = GW * w + 8 * half
                    nc.sync.dma_start(
                        out=z[b0 : b0 + 8].rearrange("b i d -> i b d"),
                        in_=zsb_t[:, :].rearrange("p (g d) -> p g d", d=D),
                    )

            # software pipeline: stage-1 of window w+1 is emitted before
            # stage-2 of window w, so the Pool engine never stalls inside a
            # stage-2 gather waiting for its own window's stage-1 transfers.
            emit_front(0)
            for w in range(N_WIN):
                if w + 1 < N_WIN:
                    emit_front(w + 1)
                emit_back(w)

    # Bacc finalize (register alloc, nop fusion, auto GPSIMD library loads,
    # ISA lowering) -- must run before the wait-splitting workaround so its
    # nop fusion can't re-merge the split waits.
    nc.compile()
    _split_sync_waits(nc)
    return nc


# ---------------------------------------------------------------------------
# Host-side packing
# ---------------------------------------------------------------------------
def _pack_m(minv32: np.ndarray) -> np.ndarray:
    """Stage-1 source layout (single-stripe: HW ucode does not support elems
    spanning rank stripes): column c of M lives contiguously on partition
    c%128 at bytes [(c//128)*2048, +2048), row-major along the column:
    M_pack[p, 1024*q + r] = bf16(M[r, 128*q + p]).
    """
    mb = np.asarray(minv32, dtype=ml_dtypes.bfloat16)
    return np.ascontiguousarray(
        mb.T.reshape(8, 128, 1024).transpose(1, 0, 2).reshape(128, 8192)
    )


def _wrap_idx512(flat: np.ndarray) -> np.ndarray:
    """[512] index list -> [128, 32] int16 wrapped layout: each 16-partition
    group holds the full list with index n = 16*s + (p % 16) at [p, s]."""
    w = flat.astype(np.int16).reshape(32, 16)  # [s, t]
    return np.ascontiguousarray(np.tile(w.T, (8, 1)))  # [128, 32]


def _pack_indices(idx_core: np.ndarray):
    """Per-window stage-1/stage-2 index tensors, [N_WIN, 128, 128] int16.
    Each window = 4 gather calls x 512 idxs (4 graphs per call); call c's
    wrapped indices occupy columns [32c, 32c+32)."""
    idx1 = np.empty((N_WIN, 128, 128), np.int16)
    idx2 = np.empty((N_WIN, 128, 128), np.int16)
    for w in range(N_WIN):
        blk = idx_core[GW * w : GW * (w + 1)].astype(np.int64)  # [16, 128]
        b_loc = np.arange(GW, dtype=np.int64)[:, None]
        t1 = blk  # column ids
        # stage-2 row id in C's rank-stripe space:
        # rank = 32*(b//4) + 4*(idx//128) + b%4 ; t = 128*rank + idx%128
        t2 = 128 * (32 * (b_loc // 4) + 4 * (blk // 128) + b_loc % 4) + blk % 128
        assert t2.max() < 16384
        for cc in range(4):
            f1 = t1[4 * cc : 4 * cc + 4].reshape(-1)  # [512]
            f2 = t2[4 * cc : 4 * cc + 4].reshape(-1)
            idx1[w, :, 32 * cc : 32 * (cc + 1)] = _wrap_idx512(f1)
            idx2[w, :, 32 * cc : 32 * (cc + 1)] = _wrap_idx512(f2)
    return idx1, idx2


# ---------------------------------------------------------------------------
# Cached compile+run (adapted from concourse.bass2jax.run_bass_via_pjrt so the
# jitted executable survives across kernel() calls).
# ---------------------------------------------------------------------------
_CACHE: dict = {}


def _get_runner():
    if "runner" in _CACHE:
        return _CACHE["runner"]

    import jax
    from jax.sharding import Mesh, PartitionSpec
    from jax.experimental.shard_map import shard_map
    from concourse.bass2jax import (
        _bass_exec_p,
        install_neuronx_cc_hook,
        partition_id_tensor,
    )

    install_neuronx_cc_hook()
    nc = build_program()

    partition_name = nc.partition_id_tensor.name if nc.partition_id_tensor else None
    in_names: list = []
    out_names: list = []
    out_avals: list = []
    out_shapes: list = []
    for alloc in nc.m.functions[0].allocations:
        if not isinstance(alloc, mybir.MemoryLocationSet):
            continue
        name = alloc.memorylocations[0].name
        if alloc.kind == "ExternalInput":
            if name != partition_name:
                in_names.append(name)
        elif alloc.kind == "ExternalOutput":
            shape = tuple(alloc.tensor_shape)
            dtype = mybir.dt.np(alloc.dtype)
            out_names.append(name)
            out_avals.append(jax.core.ShapedArray(shape, dtype))
            out_shapes.append((shape, dtype))
    n_params = len(in_names)
    n_outs = len(out_names)
    all_in_names = list(in_names) + list(out_names)
    if partition_name is not None:
        all_in_names.append(partition_name)

    def _body(*args):
        operands = list(args)
        if partition_name is not None:
            operands.append(partition_id_tensor())
        outs = _bass_exec_p.bind(
            *operands,
            out_avals=tuple(out_avals),
            in_names=tuple(all_in_names),
            out_names=tuple(out_names),
            lowering_input_output_aliases=(),
            sim_require_finite=True,
            sim_require_nnan=True,
            nc=nc,
        )
        return tuple(outs)

    try:
        devices = jax.devices("axon")[:N_CORES]
    except RuntimeError:
        devices = jax.devices()[:N_CORES]
    mesh = Mesh(np.asarray(devices), ("core",))
    donate = tuple(range(n_params, n_params + n_outs))
    sharded = jax.jit(
        shard_map(
            _body,
            mesh=mesh,
            in_specs=(PartitionSpec("core"),) * (n_params + n_outs),
            out_specs=(PartitionSpec("core"),) * n_outs,
            check_rep=False,
        ),
        donate_argnums=donate,
        keep_unused=True,
    )

    def run(in_maps):
        concat_in = [
            np.concatenate([in_maps[c][k] for c in range(N_CORES)], axis=0)
            for k in in_names
        ]
        concat_zeros = [
            np.zeros((N_CORES * s[0], *s[1:]), dt) for (s, dt) in out_shapes
        ]
        out_arrs = sharded(*concat_in, *concat_zeros)
        return [
            {
                k: np.asarray(out_arrs[i]).reshape(N_CORES, *out_shapes[i][0])[c]
                for i, k in enumerate(out_names)
            }
            for c in range(N_CORES)
        ]

    _CACHE["runner"] = run
    _CACHE["nc"] = nc
    return run


def build_in_maps(A, mu, logstd, eps, node_index):
    m64 = np.linalg.inv(np.eye(N_GLOBAL, dtype=np.float64) - np.asarray(A, np.float64).T)
    minv32 = m64.astype(np.float32)
    mpack = _pack_m(minv32).view(ml_dtypes.bfloat16)

    mu = np.ascontiguousarray(np.asarray(mu, np.float32))
    logstd = np.ascontiguousarray(np.asarray(logstd, np.float32))
    eps = np.ascontiguousarray(np.asarray(eps, np.float32))
    nidx = np.asarray(node_index)

    in_maps = []
    for c in range(N_CORES):
        r0 = ROWS_CORE * c
        sl = slice(r0, r0 + ROWS_CORE)
        i1, i2 = _pack_indices(nidx[B_CORE * c : B_CORE * (c + 1)])
        in_maps.append(
            {
                "mu": mu[sl].reshape(N_WIN, 128, EW_FREE),
                "logstd": logstd[sl].reshape(N_WIN, 128, EW_FREE),
                "eps": eps[sl].reshape(N_WIN, 128, EW_FREE),
                "mpack": mpack,
                "idx1": i1,
                "idx2": i2,
            }
        )
    return in_maps


def kernel(A, mu, logstd, eps, node_index):
    run = _get_runner()
    in_maps = build_in_maps(A, mu, logstd, eps, node_index)
    results = run(in_maps)

    sigma = np.empty((B * N_PER, D), np.float32)
    zout = np.empty((B * N_PER, D), np.float32)
    for c in range(N_CORES):
        r0 = ROWS_CORE * c
        sigma[r0 : r0 + ROWS_CORE] = results[c]["sigma"].reshape(ROWS_CORE, D)
        zout[r0 : r0 + ROWS_CORE] = results[c]["z"].reshape(ROWS_CORE, D)
    return (sigma, zout)



# revision 3
# speedup vs baseline: 2.9833x; 2.9833x over previous
"""Trainium2 Bass kernel for nn_CausalLayer (gnn_message_passing).

Reference computation:
    sigma = mu + eps * exp(logstd)                      [B*N_PER, D]
    M     = pinv(I - A.T)                               [N, N]  (well-conditioned -> inv)
    sub_b = M[idx_b, :][:, idx_b]                       [128, 128] per graph
    z_b   = sub_b @ sigma_b                             [128, D] per graph
    return (sigma, z)

Distribution: data-parallel over graphs, 256 graphs per core x 8 cores.
M is inverted once on the host; its transpose Mt (bf16) is replicated to
every core's DRAM (not SBUF -- DRAM-source gathers are far cheaper here).

Device pipeline per core, window = 8 graphs (32 windows):
  - elementwise sigma stream on [128, 512] f32 tiles laid out [i, (g, d)]
    so the per-graph matmul rhs is a direct column slice (no rearrange)
  - per graph g:
      S1: DRAM-source dma_gather of Mt rows idx_b (= columns of M),
          transpose=True -> C[p, m, i] = M[128m+p, idx_i]  [128, 1024] bf16
      S2: SBUF-source dma_gather from C with the same index list
          (token/rank addressing makes t == row id) ->
          sub_t[i, j] = M[idx_j, idx_i]  [128, 128] bf16  (matmul lhsT)
      MM: PSUM z_g = sub_t.T @ sigb[:, 64g:64g+64]
  - PSUM -> SBUF copy, z window store

Gather instructions are spread across Pool/DVE/Act engines (the v1 cost
model charges max-operand-size * engine cycle on the issuing engine);
plain DMAs ride on SP.
"""

import sys

for _p in ("/opt/trn_rl_repo",):
    if _p not in sys.path:
        sys.path.insert(0, _p)

import numpy as np
import ml_dtypes

import concourse.bass as bass
import concourse.mybir as mybir
from concourse import ap_utils
from concourse import bacc
from concourse.bass import MemorySpace
from concourse.tile import TileContext

N_GLOBAL = 1024
B = 2048
N_PER = 128
D = 64
N_CORES = 8
B_CORE = B // N_CORES            # 256 graphs per core
ROWS_CORE = B_CORE * N_PER       # 32768 rows per core
GW = 8                           # graphs per window
N_WIN = B_CORE // GW             # 32 windows per core
EW_FREE = GW * D                 # 512 f32 per partition per window tile


# ---------------------------------------------------------------------------
# Workaround: this walrus build accepts only one sync-wait per instruction
# (setupSyncWait "Too many sync wait commands").  After Tile scheduling,
# split any instruction's excess waits onto preceding NoOps on its engine --
# the engine blocks on the NoOps' waits first, so semantics are unchanged.
# ---------------------------------------------------------------------------
MAX_WAITS_PER_INST = 1


def _split_sync_waits(nc: bass.Bass, max_waits: int = MAX_WAITS_PER_INST):
    for fn in nc.m.functions:
        for blk in fn.blocks:
            insts = list(blk.instructions)
            out = []
            changed = False
            for inst in insts:
                si = inst.sync_info
                waits = list(si.on_wait) if si is not None and si.on_wait else []
                if len(waits) > max_waits:
                    changed = True
                    keep = waits[-max_waits:]
                    extra = waits[: len(waits) - max_waits]
                    for k, w in enumerate(extra):
                        nop = mybir.InstNoOp(
                            name=f"{inst.name}-ws{k}",
                            engine=inst.engine,
                            ins=[],
                            outs=[],
                            sync_info=mybir.SyncInfo(on_wait=[w], on_update=[]),
                        )
                        nc.register_instruction(nop)
                        out.append(nop)
                    inst.sync_info = mybir.SyncInfo(
                        on_wait=keep,
                        on_update=list(si.on_update) if si.on_update else [],
                    )
                out.append(inst)
            if changed:
                blk.instructions = out


# ---------------------------------------------------------------------------
# dma_gather emitted via an arbitrary engine wrapper (non-prepare_only,
# static num_idxs).  Mirrors BassGpSimd.dma_gather's body; `eng` is any
# BassEngine, so the instruction (and its modeled cost) lands on that engine.
# ---------------------------------------------------------------------------
def _exact_div(a, b):
    assert a % b == 0
    return a // b


def dma_gather_on(
    eng,
    out_ap,
    in_ap,
    idxs_ap,
    num_idxs,
    elem_size,
    transpose=True,
    queue_num=0,
    sbuf_tokens_per_rank=0,
    sbuf_free_dim_per_rank=0,
    sbuf_free_dim_pad_per_rank=0,
    sbuf_byte_offset=0,
):
    assert idxs_ap.dtype == mybir.dt.int16
    src_is_sbuf = in_ap.space == MemorySpace.SBUF
    if src_is_sbuf and in_ap.dtype != out_ap.dtype:
        in_ap = in_ap.bitcast(out_ap.dtype)
    assert in_ap.dtype == out_ap.dtype
    if transpose:
        assert mybir.dt.size(in_ap.dtype) <= 2
    elem_size_bytes = elem_size * mybir.dt.size(in_ap.dtype)
    assert elem_size_bytes > 0 and elem_size_bytes % 256 == 0

    if src_is_sbuf:
        assert transpose
        assert 0 < sbuf_tokens_per_rank <= 128 and sbuf_tokens_per_rank.bit_count() == 1
        assert 0 <= sbuf_free_dim_pad_per_rank < sbuf_free_dim_per_rank
        assert sbuf_byte_offset >= 0
        elem_step = elem_size
    else:
        assert in_ap.space == MemorySpace.DRAM
        assert ap_utils.ap_is_contiguous(in_ap.ap[1:])
        elem_step = elem_size

    assert ap_utils.ap_is_contiguous(out_ap.ap[1:])
    assert ap_utils.ap_is_contiguous(idxs_ap.ap[1:])

    if transpose:
        assert num_idxs % 128 == 0
        assert out_ap.ap[0][1] * out_ap.ap[1][1] == elem_size
        if not src_is_sbuf:
            assert in_ap.ap[-1][1] == elem_size
        assert out_ap.ap[-1][1] == num_idxs

    stride_bytes_256 = 0
    if not src_is_sbuf:
        assert in_ap.ap[0][0] == elem_step
        stride_bytes = elem_step * mybir.dt.size(in_ap.dtype)
        stride_bytes_256 = _exact_div(stride_bytes, 256)
        assert stride_bytes_256 < 256

    _in_ap = (
        [eng.lower_ap(in_ap)]
        if src_is_sbuf
        else eng.lower_ap_dma(in_ap, for_custom_bir_dma=True)
    )
    _idxs_ap = eng.lower_ap(idxs_ap)
    _out_ap = eng.lower_ap(out_ap)
    return eng.add_instruction(
        mybir.InstDMAGatherAnt(
            name=eng.bass.get_next_instruction_name(),
            ins=[*_in_ap, _idxs_ap, eng.lower_val_access(eng.to_reg(num_idxs))],
            outs=[_out_ap],
            transpose=transpose,
            num_idxs=num_idxs,
            elem_size=elem_size,
            stride_bytes_256=stride_bytes_256,
            gen_mode=0,
            single_packet=True,
            queue_num=queue_num,
            sbuf_tokens_per_rank=sbuf_tokens_per_rank,
            sbuf_free_dim_per_rank=sbuf_free_dim_per_rank,
            sbuf_free_dim_pad_per_rank=sbuf_free_dim_pad_per_rank,
            sbuf_byte_offset=sbuf_byte_offset,
        )
    )


# ---------------------------------------------------------------------------
# Device program (identical on all 8 cores; all per-core data arrives as
# DRAM inputs).
# ---------------------------------------------------------------------------
# Gathers must run on Pool: walrus codegen rejects DMAGatherAnt on any
# other engine ("Instruction engine check failed").
S1_ENGS = ["pool"] * 8
S2_ENGS = ["pool"] * 8
QNUM = {"pool": 0, "dve": 1, "act": 2}


def build_program() -> bass.Bass:
    nc = bacc.Bacc(num_swdge_queues=3, dynamic_dma_scratch_size=32768)
    f32 = mybir.dt.float32
    bf16 = mybir.dt.bfloat16
    i16 = mybir.dt.int16

    mt = nc.dram_tensor("mt", [N_GLOBAL, N_GLOBAL], bf16, kind="ExternalInput")
    mu = nc.dram_tensor("mu", [N_WIN, 128, EW_FREE], f32, kind="ExternalInput")
    ls = nc.dram_tensor("logstd", [N_WIN, 128, EW_FREE], f32, kind="ExternalInput")
    eps = nc.dram_tensor("eps", [N_WIN, 128, EW_FREE], f32, kind="ExternalInput")
    idx = nc.dram_tensor("idx", [N_WIN, 128, GW * 8], i16, kind="ExternalInput")

    sigma = nc.dram_tensor("sigma", [N_WIN, 128, EW_FREE], f32, kind="ExternalOutput")
    z = nc.dram_tensor("z", [N_WIN, 128, EW_FREE], f32, kind="ExternalOutput")

    with TileContext(nc) as tc:
        with (
            tc.tile_pool(name="ew", bufs=2) as ewp,
            tc.tile_pool(name="idxp", bufs=2) as idxp,
            tc.tile_pool(name="cg", bufs=6) as cgp,
            tc.tile_pool(name="sub", bufs=6) as subp,
            tc.tile_pool(name="zsb", bufs=2) as zsbp,
            tc.tile_pool(name="zps", bufs=4, space="PSUM") as zpsp,
        ):
            eng = {"pool": nc.gpsimd, "dve": nc.vector, "act": nc.scalar}

            for w in range(N_WIN):
                mu_t = ewp.tile([128, EW_FREE], f32, tag="mu")
                ls_t = ewp.tile([128, EW_FREE], f32, tag="ls")
                eps_t = ewp.tile([128, EW_FREE], f32, tag="eps")
                nc.sync.dma_start(out=mu_t[:, :], in_=mu[w])
                nc.sync.dma_start(out=ls_t[:, :], in_=ls[w])
                nc.sync.dma_start(out=eps_t[:, :], in_=eps[w])
                idx_t = idxp.tile([128, GW * 8], i16, tag="idx")
                nc.sync.dma_start(out=idx_t[:, :], in_=idx[w])

                tmp_t = ewp.tile([128, EW_FREE], f32, tag="tmp")
                sig_t = ewp.tile([128, EW_FREE], f32, tag="sig")
                nc.scalar.activation(
                    tmp_t[:, :], ls_t[:, :], mybir.ActivationFunctionType.Exp
                )
                nc.vector.tensor_mul(tmp_t[:, :], tmp_t[:, :], eps_t[:, :])
                nc.vector.tensor_add(sig_t[:, :], tmp_t[:, :], mu_t[:, :])
                nc.sync.dma_start(out=sigma[w], in_=sig_t[:, :])

                sigb_t = ewp.tile([128, EW_FREE], bf16, tag="sigb")
                nc.vector.tensor_copy(sigb_t[:, :], sig_t[:, :])

                zps_t = zpsp.tile([128, EW_FREE], f32)
                for g in range(GW):
                    e1 = S1_ENGS[g]
                    e2 = S2_ENGS[g]
                    c_t = cgp.tile([128, 1024], bf16, tag="c")
                    dma_gather_on(
                        eng[e1],
                        out_ap=c_t[:, :].rearrange("p (m i) -> p m i", m=8),
                        in_ap=mt[:, :],
                        idxs_ap=idx_t[:, 8 * g : 8 * (g + 1)],
                        num_idxs=128,
                        elem_size=1024,
                        transpose=True,
                        queue_num=QNUM[e1],
                    )
                    sub_t = subp.tile([128, 128], bf16, tag="sub")
                    dma_gather_on(
                        eng[e2],
                        out_ap=sub_t[:, :].rearrange("p (o i) -> p o i", o=1),
                        in_ap=c_t[:, :],
                        idxs_ap=idx_t[:, 8 * g : 8 * (g + 1)],
                        num_idxs=128,
                        elem_size=128,
                        transpose=True,
                        queue_num=QNUM[e2],
                        sbuf_tokens_per_rank=128,
                        sbuf_free_dim_per_rank=256,
                        sbuf_free_dim_pad_per_rank=0,
                        sbuf_byte_offset=0,
                    )
                    nc.tensor.matmul(
                        zps_t[:, D * g : D * (g + 1)],
                        sub_t[:, :],
                        sigb_t[:, D * g : D * (g + 1)],
                        start=True,
                        stop=True,
                    )
                zsb_t = zsbp.tile([128, EW_FREE], f32)
                nc.scalar.copy(out=zsb_t[:, :], in_=zps_t[:, :])
                nc.sync.dma_start(out=z[w], in_=zsb_t[:, :])

    nc.compile()
    _split_sync_waits(nc)
    return nc


# ---------------------------------------------------------------------------
# Host-side packing
# ---------------------------------------------------------------------------
def _wrap_idx128(flat: np.ndarray) -> np.ndarray:
    """[128] index list -> [128, 8] int16 wrapped layout: each 16-partition
    group holds the full list with index n = 16*s + (p % 16) at [p, s]."""
    w = flat.astype(np.int16).reshape(8, 16)  # [s, t]
    return np.ascontiguousarray(np.tile(w.T, (8, 1)))  # [128, 8]


def _pack_indices(idx_core: np.ndarray) -> np.ndarray:
    """[B_CORE, 128] node ids -> [N_WIN, 128, GW*8] int16 wrapped per graph."""
    out = np.empty((N_WIN, 128, GW * 8), np.int16)
    for w in range(N_WIN):
        for g in range(GW):
            out[w, :, 8 * g : 8 * (g + 1)] = _wrap_idx128(idx_core[GW * w + g])
    return out


def _pack_ew(x: np.ndarray) -> np.ndarray:
    """[ROWS_CORE, D] row-major -> [N_WIN, 128, GW*D]: T[w][i, g*D+d] =
    x[N_PER*GW*w + N_PER*g + i, d]."""
    return np.ascontiguousarray(
        x.reshape(N_WIN, GW, N_PER, D).transpose(0, 2, 1, 3).reshape(N_WIN, 128, GW * D)
    )


def _unpack_ew(t: np.ndarray) -> np.ndarray:
    return np.ascontiguousarray(
        t.reshape(N_WIN, 128, GW, D).transpose(0, 2, 1, 3).reshape(ROWS_CORE, D)
    )


# ---------------------------------------------------------------------------
# Cached compile+run (adapted from concourse.bass2jax.run_bass_via_pjrt so the
# jitted executable survives across kernel() calls).
# ---------------------------------------------------------------------------
_CACHE: dict = {}


def _get_runner():
    if "runner" in _CACHE:
        return _CACHE["runner"]

    import jax
    from jax.sharding import Mesh, PartitionSpec
    from jax.experimental.shard_map import shard_map
    from concourse.bass2jax import (
        _bass_exec_p,
        install_neuronx_cc_hook,
        partition_id_tensor,
    )

    install_neuronx_cc_hook()
    nc = build_program()

    partition_name = nc.partition_id_tensor.name if nc.partition_id_tensor else None
    in_names: list = []
    out_names: list = []
    out_avals: list = []
    out_shapes: list = []
    for alloc in nc.m.functions[0].allocations:
        if not isinstance(alloc, mybir.MemoryLocationSet):
            continue
        name = alloc.memorylocations[0].name
        if alloc.kind == "ExternalInput":
            if name != partition_name:
                in_names.append(name)
        elif alloc.kind == "ExternalOutput":
            shape = tuple(alloc.tensor_shape)
            dtype = mybir.dt.np(alloc.dtype)
            out_names.append(name)
            out_avals.append(jax.core.ShapedArray(shape, dtype))
            out_shapes.append((shape, dtype))
    n_params = len(in_names)
    n_outs = len(out_names)
    all_in_names = list(in_names) + list(out_names)
    if partition_name is not None:
        all_in_names.append(partition_name)

    def _body(*args):
        operands = list(args)
        if partition_name is not None:
            operands.append(partition_id_tensor())
        outs = _bass_exec_p.bind(
            *operands,
            out_avals=tuple(out_avals),
            in_names=tuple(all_in_names),
            out_names=tuple(out_names),
            lowering_input_output_aliases=(),
            sim_require_finite=True,
            sim_require_nnan=True,
            nc=nc,
        )
        return tuple(outs)

    try:
        devices = jax.devices("axon")[:N_CORES]
    except RuntimeError:
        devices = jax.devices()[:N_CORES]
    mesh = Mesh(np.asarray(devices), ("core",))
    donate = tuple(range(n_params, n_params + n_outs))
    sharded = jax.jit(
        shard_map(
            _body,
            mesh=mesh,
            in_specs=(PartitionSpec("core"),) * (n_params + n_outs),
            out_specs=(PartitionSpec("core"),) * n_outs,
            check_rep=False,
        ),
        donate_argnums=donate,
        keep_unused=True,
    )

    def run(in_maps):
        concat_in = [
            np.concatenate([in_maps[c][k] for c in range(N_CORES)], axis=0)
            for k in in_names
        ]
        concat_zeros = [
            np.zeros((N_CORES * s[0], *s[1:]), dt) for (s, dt) in out_shapes
        ]
        out_arrs = sharded(*concat_in, *concat_zeros)
        return [
            {
                k: np.asarray(out_arrs[i]).reshape(N_CORES, *out_shapes[i][0])[c]
                for i, k in enumerate(out_names)
            }
            for c in range(N_CORES)
        ]

    _CACHE["runner"] = run
    _CACHE["nc"] = nc
    return run


def build_in_maps(A, mu, logstd, eps, node_index):
    m64 = np.linalg.inv(np.eye(N_GLOBAL, dtype=np.float64) - np.asarray(A, np.float64).T)
    minv32 = m64.astype(np.float32)
    mt = np.ascontiguousarray(minv32.T).astype(ml_dtypes.bfloat16)

    mu = np.ascontiguousarray(np.asarray(mu, np.float32))
    logstd = np.ascontiguousarray(np.asarray(logstd, np.float32))
    eps = np.ascontiguousarray(np.asarray(eps, np.float32))
    nidx = np.asarray(node_index)

    in_maps = []
    for c in range(N_CORES):
        r0 = ROWS_CORE * c
        sl = slice(r0, r0 + ROWS_CORE)
        in_maps.append(
            {
                "mt": mt,
                "mu": _pack_ew(mu[sl]),
                "logstd": _pack_ew(logstd[sl]),
                "eps": _pack_ew(eps[sl]),
                "idx": _pack_indices(nidx[B_CORE * c : B_CORE * (c + 1)]),
            }
        )
    return in_maps


def kernel(A, mu, logstd, eps, node_index):
    run = _get_runner()
    in_maps = build_in_maps(A, mu, logstd, eps, node_index)
    results = run(in_maps)

    sigma = np.empty((B * N_PER, D), np.float32)
    zout = np.empty((B * N_PER, D), np.float32)
    for c in range(N_CORES):
        r0 = ROWS_CORE * c
        sigma[r0 : r0 + ROWS_CORE] = _unpack_ew(results[c]["sigma"])
        zout[r0 : r0 + ROWS_CORE] = _unpack_ew(results[c]["z"])
    return (sigma, zout)


# revision 8
# speedup vs baseline: 5.8194x; 1.9507x over previous
"""Trainium2 Bass kernel for nn_CausalLayer (gnn_message_passing).

Reference computation:
    sigma = mu + eps * exp(logstd)                      [B*N_PER, D]
    M     = pinv(I - A.T)                               [N, N]  (well-conditioned -> inv)
    sub_b = M[idx_b, :][:, idx_b]                       [128, 128] per graph
    z_b   = sub_b @ sigma_b                             [128, D] per graph
    return (sigma, z)

Distribution: data-parallel over graphs, 256 graphs per core x 8 cores.
M is inverted once on the host; its transpose Mt (bf16) is replicated to
every core's DRAM (not SBUF -- DRAM-source gathers are far cheaper here).

Device pipeline per core, window = 8 graphs (32 windows):
  - elementwise sigma stream on [128, 512] f32 tiles laid out [i, (g, d)]
    so the per-graph matmul rhs is a direct column slice (no rearrange)
  - M is packed fp8-e4m3 (scaled by 2^4), TWO CONSECUTIVE ROWS PER 16-BIT
    CELL: the gathers are byte movers on 16-bit units, so declaring the
    fp8 pairs as bf16 halves both gather footprints (gather cost here is
    operand-footprint-bound, not descriptor-bound).
  - per graph g:
      S1: DRAM-source dma_gather of packed-Mt rows idx_b, transpose=True
          -> C8 cell[p, m, i] = (M8[2(128m+p), idx_i], M8[2(128m+p)+1, idx_i])
      S2: SBUF-source dma_gather from C8 with t2 = idx_j >> 1 ->
          sub8 cell[i, j] = (M8[r0_j, idx_i], M8[r0_j+1, idx_i]), r0 = idx_j & ~1
      MM: two fp8 matmuls (even/odd byte-strided lhsT views) ->
          PSUM z_e, z_o; the 2^-4 unscale is folded into sigb.
  - per window: DVE parity select z = z_e + par*(z_o - z_e) (par is a
    host-packed f32 mask tile), written straight to SBUF, then z store.

Gathers must run on Pool (walrus rejects other engines); plain DMAs are
split between SP and Act.
"""

import sys

for _p in ("/opt/trn_rl_repo",):
    if _p not in sys.path:
        sys.path.insert(0, _p)

import numpy as np
import ml_dtypes

import concourse.bass as bass
import concourse.mybir as mybir
from concourse import ap_utils
from concourse import bacc
from concourse.bass import MemorySpace
from concourse.tile import TileContext

N_GLOBAL = 1024
B = 2048
N_PER = 128
D = 64
N_CORES = 8
B_CORE = B // N_CORES            # 256 graphs per core
ROWS_CORE = B_CORE * N_PER       # 32768 rows per core
GW = 8                           # graphs per window
N_WIN = B_CORE // GW             # 32 windows per core
EW_FREE = GW * D                 # 512 f32 per partition per window tile


# ---------------------------------------------------------------------------
# Workaround: this walrus build accepts only one sync-wait per instruction
# (setupSyncWait "Too many sync wait commands").  After Tile scheduling,
# split any instruction's excess waits onto preceding NoOps on its engine --
# the engine blocks on the NoOps' waits first, so semantics are unchanged.
# ---------------------------------------------------------------------------
MAX_WAITS_PER_INST = 1


def _split_sync_waits(nc: bass.Bass, max_waits: int = MAX_WAITS_PER_INST):
    for fn in nc.m.functions:
        for blk in fn.blocks:
            insts = list(blk.instructions)
            out = []
            changed = False
            for inst in insts:
                si = inst.sync_info
                waits = list(si.on_wait) if si is not None and si.on_wait else []
                if len(waits) > max_waits:
                    changed = True
                    keep = waits[-max_waits:]
                    extra = waits[: len(waits) - max_waits]
                    for k, w in enumerate(extra):
                        nop = mybir.InstNoOp(
                            name=f"{inst.name}-ws{k}",
                            engine=inst.engine,
                            ins=[],
                            outs=[],
                            sync_info=mybir.SyncInfo(on_wait=[w], on_update=[]),
                        )
                        nc.register_instruction(nop)
                        out.append(nop)
                    inst.sync_info = mybir.SyncInfo(
                        on_wait=keep,
                        on_update=list(si.on_update) if si.on_update else [],
                    )
                out.append(inst)
            if changed:
                blk.instructions = out


# ---------------------------------------------------------------------------
# dma_gather emitted via an arbitrary engine wrapper (non-prepare_only,
# static num_idxs).  Mirrors BassGpSimd.dma_gather's body; `eng` is any
# BassEngine, so the instruction (and its modeled cost) lands on that engine.
# ---------------------------------------------------------------------------
def _exact_div(a, b):
    assert a % b == 0
    return a // b


def dma_gather_on(
    eng,
    out_ap,
    in_ap,
    idxs_ap,
    num_idxs,
    elem_size,
    transpose=True,
    queue_num=0,
    sbuf_tokens_per_rank=0,
    sbuf_free_dim_per_rank=0,
    sbuf_free_dim_pad_per_rank=0,
    sbuf_byte_offset=0,
):
    assert idxs_ap.dtype == mybir.dt.int16
    src_is_sbuf = in_ap.space == MemorySpace.SBUF
    if src_is_sbuf and in_ap.dtype != out_ap.dtype:
        in_ap = in_ap.bitcast(out_ap.dtype)
    assert in_ap.dtype == out_ap.dtype
    if transpose:
        assert mybir.dt.size(in_ap.dtype) <= 2
    elem_size_bytes = elem_size * mybir.dt.size(in_ap.dtype)
    assert elem_size_bytes > 0 and elem_size_bytes % 256 == 0

    if src_is_sbuf:
        assert transpose
        assert 0 < sbuf_tokens_per_rank <= 128 and sbuf_tokens_per_rank.bit_count() == 1
        assert 0 <= sbuf_free_dim_pad_per_rank < sbuf_free_dim_per_rank
        assert sbuf_byte_offset >= 0
        elem_step = elem_size
    else:
        assert in_ap.space == MemorySpace.DRAM
        assert ap_utils.ap_is_contiguous(in_ap.ap[1:])
        elem_step = elem_size

    assert ap_utils.ap_is_contiguous(out_ap.ap[1:])
    assert ap_utils.ap_is_contiguous(idxs_ap.ap[1:])

    if transpose:
        assert num_idxs % 128 == 0
        assert out_ap.ap[0][1] * out_ap.ap[1][1] == elem_size
        if not src_is_sbuf:
            assert in_ap.ap[-1][1] == elem_size
        assert out_ap.ap[-1][1] == num_idxs

    stride_bytes_256 = 0
    if not src_is_sbuf:
        assert in_ap.ap[0][0] == elem_step
        stride_bytes = elem_step * mybir.dt.size(in_ap.dtype)
        stride_bytes_256 = _exact_div(stride_bytes, 256)
        assert stride_bytes_256 < 256

    _in_ap = (
        [eng.lower_ap(in_ap)]
        if src_is_sbuf
        else eng.lower_ap_dma(in_ap, for_custom_bir_dma=True)
    )
    _idxs_ap = eng.lower_ap(idxs_ap)
    _out_ap = eng.lower_ap(out_ap)
    return eng.add_instruction(
        mybir.InstDMAGatherAnt(
            name=eng.bass.get_next_instruction_name(),
            ins=[*_in_ap, _idxs_ap, eng.lower_val_access(eng.to_reg(num_idxs))],
            outs=[_out_ap],
            transpose=transpose,
            num_idxs=num_idxs,
            elem_size=elem_size,
            stride_bytes_256=stride_bytes_256,
            gen_mode=0,
            single_packet=True,
            queue_num=queue_num,
            sbuf_tokens_per_rank=sbuf_tokens_per_rank,
            sbuf_free_dim_per_rank=sbuf_free_dim_per_rank,
            sbuf_free_dim_pad_per_rank=sbuf_free_dim_pad_per_rank,
            sbuf_byte_offset=sbuf_byte_offset,
        )
    )


# ---------------------------------------------------------------------------
# Device program (identical on all 8 cores; all per-core data arrives as
# DRAM inputs).
# ---------------------------------------------------------------------------
M_SCALE = 16.0  # fp8 pre-scale (2^4); unscale folded into sigb


def build_program() -> bass.Bass:
    nc = bacc.Bacc(num_swdge_queues=1, dynamic_dma_scratch_size=32768)
    f32 = mybir.dt.float32
    bf16 = mybir.dt.bfloat16
    fp8 = mybir.dt.float8e4
    i16 = mybir.dt.int16

    # packed fp8 Mt: row c holds bytes M8[r, c] for r = 0..1023, declared as
    # 512 bf16 cells (cell u = rows 2u, 2u+1).
    mt = nc.dram_tensor("mt", [N_GLOBAL, N_GLOBAL // 2], bf16, kind="ExternalInput")
    mu = nc.dram_tensor("mu", [N_WIN, 128, EW_FREE], f32, kind="ExternalInput")
    ls = nc.dram_tensor("logstd", [N_WIN, 128, EW_FREE], f32, kind="ExternalInput")
    eps = nc.dram_tensor("eps", [N_WIN, 128, EW_FREE], f32, kind="ExternalInput")
    idx1 = nc.dram_tensor("idx1", [N_WIN, 128, GW * 8], i16, kind="ExternalInput")
    idx2 = nc.dram_tensor("idx2", [N_WIN, 128, GW * 8], i16, kind="ExternalInput")
    par = nc.dram_tensor("par", [N_WIN, 128, EW_FREE], f32, kind="ExternalInput")

    sigma = nc.dram_tensor("sigma", [N_WIN, 128, EW_FREE], f32, kind="ExternalOutput")
    z = nc.dram_tensor("z", [N_WIN, 128, EW_FREE], f32, kind="ExternalOutput")

    with TileContext(nc) as tc:
        with (
            tc.tile_pool(name="ew", bufs=2) as ewp,
            tc.tile_pool(name="idxp", bufs=2) as idxp,
            tc.tile_pool(name="cg", bufs=6) as cgp,
            tc.tile_pool(name="sub", bufs=6) as subp,
            tc.tile_pool(name="zsb", bufs=2) as zsbp,
            tc.tile_pool(name="zpse", bufs=2, space="PSUM") as zpsep,
            tc.tile_pool(name="zpso", bufs=2, space="PSUM") as zpsop,
        ):
            for w in range(N_WIN):
                mu_t = ewp.tile([128, EW_FREE], f32, tag="mu")
                ls_t = ewp.tile([128, EW_FREE], f32, tag="ls")
                eps_t = ewp.tile([128, EW_FREE], f32, tag="eps")
                nc.sync.dma_start(out=mu_t[:, :], in_=mu[w])
                nc.sync.dma_start(out=ls_t[:, :], in_=ls[w])
                nc.sync.dma_start(out=eps_t[:, :], in_=eps[w])
                idx1_t = idxp.tile([128, GW * 8], i16, tag="idx1")
                idx2_t = idxp.tile([128, GW * 8], i16, tag="idx2")
                nc.scalar.dma_start(out=idx1_t[:, :], in_=idx1[w])
                nc.scalar.dma_start(out=idx2_t[:, :], in_=idx2[w])
                par_t = ewp.tile([128, EW_FREE], f32, tag="par")
                nc.scalar.dma_start(out=par_t[:, :], in_=par[w])

                tmp_t = ewp.tile([128, EW_FREE], f32, tag="tmp")
                sig_t = ewp.tile([128, EW_FREE], f32, tag="sig")
                nc.scalar.activation(
                    tmp_t[:, :], ls_t[:, :], mybir.ActivationFunctionType.Exp
                )
                nc.vector.tensor_mul(tmp_t[:, :], tmp_t[:, :], eps_t[:, :])
                nc.vector.tensor_add(sig_t[:, :], tmp_t[:, :], mu_t[:, :])
                nc.sync.dma_start(out=sigma[w], in_=sig_t[:, :])

                # bf16 copy with the fp8 unscale folded in
                sigb_t = ewp.tile([128, EW_FREE], bf16, tag="sigb")
                nc.vector.tensor_scalar_mul(sigb_t[:, :], sig_t[:, :], 1.0 / M_SCALE)

                zpse_t = zpsep.tile([128, EW_FREE], f32)
                zpso_t = zpsop.tile([128, EW_FREE], f32)
                for g in range(GW):
                    c_t = cgp.tile([128, 512], bf16, tag="c")
                    dma_gather_on(
                        nc.gpsimd,
                        out_ap=c_t[:, :].rearrange("p (m i) -> p m i", m=4),
                        in_ap=mt[:, :],
                        idxs_ap=idx1_t[:, 8 * g : 8 * (g + 1)],
                        num_idxs=128,
                        elem_size=512,
                        transpose=True,
                        queue_num=0,
                    )
                    sub_t = subp.tile([128, 256], fp8, tag="sub")
                    dma_gather_on(
                        nc.gpsimd,
                        out_ap=sub_t[:, :]
                        .bitcast(bf16)
                        .rearrange("p (o i) -> p o i", o=1),
                        in_ap=c_t[:, :],
                        idxs_ap=idx2_t[:, 8 * g : 8 * (g + 1)],
                        num_idxs=128,
                        elem_size=128,
                        transpose=True,
                        queue_num=0,
                        sbuf_tokens_per_rank=128,
                        sbuf_free_dim_per_rank=256,
                        sbuf_free_dim_pad_per_rank=0,
                        sbuf_byte_offset=0,
                    )
                    sub_v = sub_t[:, :].rearrange("p (j two) -> p j two", two=2)
                    nc.tensor.matmul(
                        zpse_t[:, D * g : D * (g + 1)],
                        sub_v[:, :, 0],
                        sigb_t[:, D * g : D * (g + 1)],
                        start=True,
                        stop=True,
                    )
                    nc.tensor.matmul(
                        zpso_t[:, D * g : D * (g + 1)],
                        sub_v[:, :, 1],
                        sigb_t[:, D * g : D * (g + 1)],
                        start=True,
                        stop=True,
                    )
                # z = z_e + par * (z_o - z_e); walrus allows at most one PSUM
                # operand per vector op, so stage z_e through SBUF first.
                ze_t = zsbp.tile([128, EW_FREE], f32, tag="ze")
                d_t = zsbp.tile([128, EW_FREE], f32, tag="d")
                zsb_t = zsbp.tile([128, EW_FREE], f32, tag="z")
                nc.scalar.copy(out=ze_t[:, :], in_=zpse_t[:, :])
                nc.vector.tensor_sub(d_t[:, :], zpso_t[:, :], ze_t[:, :])
                nc.vector.tensor_mul(d_t[:, :], d_t[:, :], par_t[:, :])
                nc.vector.tensor_add(zsb_t[:, :], d_t[:, :], ze_t[:, :])
                nc.scalar.dma_start(out=z[w], in_=zsb_t[:, :])

    nc.compile()
    _split_sync_waits(nc)
    return nc


# ---------------------------------------------------------------------------
# Host-side packing
# ---------------------------------------------------------------------------
def _wrap_idx128(flat: np.ndarray) -> np.ndarray:
    """[128] index list -> [128, 8] int16 wrapped layout: each 16-partition
    group holds the full list with index n = 16*s + (p % 16) at [p, s]."""
    w = flat.astype(np.int16).reshape(8, 16)  # [s, t]
    return np.ascontiguousarray(np.tile(w.T, (8, 1)))  # [128, 8]


def _pack_indices(idx_core: np.ndarray):
    """[B_CORE, 128] node ids -> (idx1, idx2, par):
    idx1 = wrapped node ids (S1: packed-Mt row gather),
    idx2 = wrapped node ids >> 1 (S2: pair-cell token/rank ids),
    par  = [N_WIN, 128, GW*D] f32 parity mask, par[w][j, g*D+d] = id & 1."""
    idx1 = np.empty((N_WIN, 128, GW * 8), np.int16)
    idx2 = np.empty((N_WIN, 128, GW * 8), np.int16)
    par = np.empty((N_WIN, 128, GW * D), np.float32)
    for w in range(N_WIN):
        for g in range(GW):
            ids = idx_core[GW * w + g].astype(np.int64)
            idx1[w, :, 8 * g : 8 * (g + 1)] = _wrap_idx128(ids)
            idx2[w, :, 8 * g : 8 * (g + 1)] = _wrap_idx128(ids >> 1)
            par[w, :, D * g : D * (g + 1)] = (ids & 1).astype(np.float32)[:, None]
    return idx1, idx2, par


def _pack_ew(x: np.ndarray) -> np.ndarray:
    """[ROWS_CORE, D] row-major -> [N_WIN, 128, GW*D]: T[w][i, g*D+d] =
    x[N_PER*GW*w + N_PER*g + i, d]."""
    return np.ascontiguousarray(
        x.reshape(N_WIN, GW, N_PER, D).transpose(0, 2, 1, 3).reshape(N_WIN, 128, GW * D)
    )


def _unpack_ew(t: np.ndarray) -> np.ndarray:
    return np.ascontiguousarray(
        t.reshape(N_WIN, 128, GW, D).transpose(0, 2, 1, 3).reshape(ROWS_CORE, D)
    )


# ---------------------------------------------------------------------------
# Cached compile+run (adapted from concourse.bass2jax.run_bass_via_pjrt so the
# jitted executable survives across kernel() calls).
# ---------------------------------------------------------------------------
_CACHE: dict = {}


def _get_runner():
    if "runner" in _CACHE:
        return _CACHE["runner"]

    import jax
    from jax.sharding import Mesh, PartitionSpec
    from jax.experimental.shard_map import shard_map
    from concourse.bass2jax import (
        _bass_exec_p,
        install_neuronx_cc_hook,
        partition_id_tensor,
    )

    install_neuronx_cc_hook()
    nc = build_program()

    partition_name = nc.partition_id_tensor.name if nc.partition_id_tensor else None
    in_names: list = []
    out_names: list = []
    out_avals: list = []
    out_shapes: list = []
    for alloc in nc.m.functions[0].allocations:
        if not isinstance(alloc, mybir.MemoryLocationSet):
            continue
        name = alloc.memorylocations[0].name
        if alloc.kind == "ExternalInput":
            if name != partition_name:
                in_names.append(name)
        elif alloc.kind == "ExternalOutput":
            shape = tuple(alloc.tensor_shape)
            dtype = mybir.dt.np(alloc.dtype)
            out_names.append(name)
            out_avals.append(jax.core.ShapedArray(shape, dtype))
            out_shapes.append((shape, dtype))
    n_params = len(in_names)
    n_outs = len(out_names)
    all_in_names = list(in_names) + list(out_names)
    if partition_name is not None:
        all_in_names.append(partition_name)

    def _body(*args):
        operands = list(args)
        if partition_name is not None:
            operands.append(partition_id_tensor())
        outs = _bass_exec_p.bind(
            *operands,
            out_avals=tuple(out_avals),
            in_names=tuple(all_in_names),
            out_names=tuple(out_names),
            lowering_input_output_aliases=(),
            sim_require_finite=True,
            sim_require_nnan=True,
            nc=nc,
        )
        return tuple(outs)

    try:
        devices = jax.devices("axon")[:N_CORES]
    except RuntimeError:
        devices = jax.devices()[:N_CORES]
    mesh = Mesh(np.asarray(devices), ("core",))
    donate = tuple(range(n_params, n_params + n_outs))
    sharded = jax.jit(
        shard_map(
            _body,
            mesh=mesh,
            in_specs=(PartitionSpec("core"),) * (n_params + n_outs),
            out_specs=(PartitionSpec("core"),) * n_outs,
            check_rep=False,
        ),
        donate_argnums=donate,
        keep_unused=True,
    )

    def run(in_maps):
        concat_in = [
            np.concatenate([in_maps[c][k] for c in range(N_CORES)], axis=0)
            for k in in_names
        ]
        concat_zeros = [
            np.zeros((N_CORES * s[0], *s[1:]), dt) for (s, dt) in out_shapes
        ]
        out_arrs = sharded(*concat_in, *concat_zeros)
        return [
            {
                k: np.asarray(out_arrs[i]).reshape(N_CORES, *out_shapes[i][0])[c]
                for i, k in enumerate(out_names)
            }
            for c in range(N_CORES)
        ]

    _CACHE["runner"] = run
    _CACHE["nc"] = nc
    return run


def build_in_maps(A, mu, logstd, eps, node_index):
    m64 = np.linalg.inv(np.eye(N_GLOBAL, dtype=np.float64) - np.asarray(A, np.float64).T)
    minv32 = m64.astype(np.float32)
    # packed fp8 Mt: mt8[c, r] = fp8(M[r, c] * M_SCALE); pairs of consecutive
    # r-bytes form the bf16-declared cells.
    mt8 = (minv32.T * M_SCALE).astype(ml_dtypes.float8_e4m3)
    mt = np.ascontiguousarray(mt8).view(np.uint16).view(ml_dtypes.bfloat16)

    mu = np.ascontiguousarray(np.asarray(mu, np.float32))
    logstd = np.ascontiguousarray(np.asarray(logstd, np.float32))
    eps = np.ascontiguousarray(np.asarray(eps, np.float32))
    nidx = np.asarray(node_index)

    in_maps = []
    for c in range(N_CORES):
        r0 = ROWS_CORE * c
        sl = slice(r0, r0 + ROWS_CORE)
        i1, i2, pv = _pack_indices(nidx[B_CORE * c : B_CORE * (c + 1)])
        in_maps.append(
            {
                "mt": mt,
                "mu": _pack_ew(mu[sl]),
                "logstd": _pack_ew(logstd[sl]),
                "eps": _pack_ew(eps[sl]),
                "idx1": i1,
                "idx2": i2,
                "par": pv,
            }
        )
    return in_maps


def kernel(A, mu, logstd, eps, node_index):
    run = _get_runner()
    in_maps = build_in_maps(A, mu, logstd, eps, node_index)
    results = run(in_maps)

    sigma = np.empty((B * N_PER, D), np.float32)
    zout = np.empty((B * N_PER, D), np.float32)
    for c in range(N_CORES):
        r0 = ROWS_CORE * c
        sigma[r0 : r0 + ROWS_CORE] = _unpack_ew(results[c]["sigma"])
        zout[r0 : r0 + ROWS_CORE] = _unpack_ew(results[c]["z"])
    return (sigma, zout)
